# revision 1
# baseline (speedup 1.0000x reference)
"""Distributed Trainium2 kernel for a causal multi-head self-attention block.

  out = softmax_causal((x@Wq+bq)(x@Wk+bk)^T / sqrt(Dh)) (x@Wv+bv) @ W_out + b_out

Sharding (8 NeuronCores, tensor-parallel over heads):
  - Each core owns 2 of the 16 heads, both batches -> 4 (batch, head) units.
  - QKV projection computed in transposed layout (features on partitions)
    from a host-pretransposed xT, so q/k land directly in [Dh, S] form.
  - Attention: scoresT = kT-tile.T @ qT (t on partitions, s free); exp with
    no max subtraction (scores ~ N(0,1), safe); causal at tile granularity
    with a triangular mask multiply on diagonal tiles; PV matmul uses
    [v | ones] so the softmax denominator falls out of PSUM column 128.
  - Attention output is normalized, PE-transposed to [Dh, s], v-bias added,
    then a per-batch AllToAll redistributes head-shards -> token-shards
    (each core owns 256 tokens of each batch). The b=0 collective overlaps
    b=1 attention; b=0 out-projection overlaps b=1 attention/collective.
  - Output projection is token-parallel with the full W_out; host
    reassembles, transposes, and adds b_out.

All matmul operands are bf16 (1 cycle/row on the PE), accumulation f32.
"""

import math
import numpy as np
import ml_dtypes

import sys

for _p in ("/opt/trn_rl_repo",):
    if _p not in sys.path:
        sys.path.insert(0, _p)

import concourse.bass as bass
import concourse.bacc as bacc
import concourse.mybir as mybir
import concourse.tile as tile
from concourse.bass_utils import run_bass_kernel_spmd

BF16 = mybir.dt.bfloat16
F32 = mybir.dt.float32
NPBF16 = ml_dtypes.bfloat16

B, S, D = 2, 2048, 2048
H, DH = 16, 128
NC = 8
HL = H // NC            # heads per core = 2
SC = 512                # s-chunk (free dim of scores matmul)
NT = S // 128           # 16 t-tiles per batch
TOKB = S // NC          # 256 tokens owned per core per batch
INV_SQRT_DH = 1.0 / math.sqrt(DH)


def build_kernel(debug=False):
    nc = bacc.Bacc("TRN2", target_bir_lowering=False, debug=False, num_devices=NC)

    xT = nc.declare_dram_parameter("xT", [B, D, S], BF16, isOutput=False)
    wq = nc.declare_dram_parameter("wq", [HL, 16, 128, 128], BF16, isOutput=False)
    wk = nc.declare_dram_parameter("wk", [HL, 16, 128, 128], BF16, isOutput=False)
    wv = nc.declare_dram_parameter("wv", [16, 128, HL * 128], BF16, isOutput=False)
    bq = nc.declare_dram_parameter("bq", [HL, 128, 1], F32, isOutput=False)
    bk = nc.declare_dram_parameter("bk", [HL, 128, 1], F32, isOutput=False)
    bv = nc.declare_dram_parameter("bv", [HL, 128, 1], F32, isOutput=False)
    w_out = nc.declare_dram_parameter("w_out", [16, 128, D], BF16, isOutput=False)
    ident = nc.declare_dram_parameter("ident", [128, 128], BF16, isOutput=False)
    maskp = nc.declare_dram_parameter("maskp", [128, 128], BF16, isOutput=False)
    out = nc.declare_dram_parameter("out", [B, D, TOKB], F32, isOutput=True)
    if debug:
        dbg_qT = nc.declare_dram_parameter("dbg_qT", [128, 2 * HL, S], BF16, isOutput=True)
        dbg_kT = nc.declare_dram_parameter("dbg_kT", [128, 2 * HL, S], BF16, isOutput=True)
        dbg_vv = nc.declare_dram_parameter("dbg_vv", [128, 2 * HL, NT, 129], BF16, isOutput=True)
        dbg_a2a = nc.declare_dram_parameter("dbg_a2a", [NC, HL, 2, 128, 128], BF16, isOutput=True)

    with tile.TileContext(nc) as tc:
        with (
            tc.tile_pool(name="wpool", bufs=1) as wpool,
            tc.tile_pool(name="xpool", bufs=2) as xpool,
            tc.tile_pool(name="qkv", bufs=1) as qkvpool,
            tc.tile_pool(name="expp", bufs=4) as expp,
            tc.tile_pool(name="small", bufs=4) as small,
            tc.tile_pool(name="wo", bufs=1) as wopool,
            tc.tile_pool(name="rcv", bufs=1) as rcvpool,
            tc.tile_pool(name="outp", bufs=2) as outp,
            tc.tile_pool(name="psum", bufs=4, space="PSUM") as psum,
            tc.tile_pool(name="dram", bufs=1, space="DRAM") as dram,
        ):
            # ---- constants / weights needed first ----
            wq_t = wpool.tile([128, HL, 16, 128], BF16, tag="wq")
            wk_t = wpool.tile([128, HL, 16, 128], BF16, tag="wk")
            wv_t = wpool.tile([128, 16, HL * 128], BF16, tag="wv")
            bq_t = wpool.tile([128, HL, 1], F32, tag="bq")
            bk_t = wpool.tile([128, HL, 1], F32, tag="bk")
            bv_t = wpool.tile([128, HL, 1], F32, tag="bv")
            id_t = wpool.tile([128, 128], BF16, tag="ident")
            mask_t = wpool.tile([128, 128], BF16, tag="maskp")
            nc.gpsimd.dma_start(wq_t[:], wq[:].rearrange("h d p m -> p h d m"))
            nc.gpsimd.dma_start(wk_t[:], wk[:].rearrange("h d p m -> p h d m"))
            nc.gpsimd.dma_start(wv_t[:], wv[:].rearrange("d p m -> p d m"))
            nc.gpsimd.dma_start(bq_t[:], bq[:].rearrange("h p m -> p h m"))
            nc.gpsimd.dma_start(bk_t[:], bk[:].rearrange("h p m -> p h m"))
            nc.gpsimd.dma_start(bv_t[:], bv[:].rearrange("h p m -> p h m"))
            nc.gpsimd.dma_start(id_t[:], ident[:])
            nc.gpsimd.dma_start(mask_t[:], maskp[:])

            # ---- persistent activations ----
            qT = qkvpool.tile([128, 2 * HL, S], BF16, tag="qT")
            kT = qkvpool.tile([128, 2 * HL, S], BF16, tag="kT")
            vv = qkvpool.tile([128, 2 * HL, NT, 129], BF16, tag="vv")
            nc.gpsimd.memset(vv[:, :, :, 128:129], 1.0)

            # per-batch A2A bounce buffers (DRAM)
            a2a_in = [
                dram.tile([NC, HL, 2, 128, 128], BF16, tag=f"a2a_in{b}",
                          name=f"a2a_in{b}")
                for b in range(B)
            ]
            a2a_out = [
                dram.tile([NC, HL, 2, 128, 128], BF16, tag=f"a2a_out{b}",
                          name=f"a2a_out{b}")
                for b in range(B)
            ]

            def qkv_phase(b):
                for tcn in range(S // SC):
                    xt = xpool.tile([128, 16, SC], BF16, tag="xt",
                                    name=f"xt_{b}_{tcn}")
                    for g in range(4):
                        nc.sync.dma_start(
                            xt[:, 4 * g : 4 * g + 4],
                            xT[b, 4 * g * 128 : (4 * g + 4) * 128,
                               tcn * SC : (tcn + 1) * SC].rearrange(
                                "(n p) m -> p n m", p=128
                            ),
                        )
                    for hl in range(HL):
                        u = b * HL + hl
                        for w_t, b_t, dst in ((wq_t, bq_t, qT), (wk_t, bk_t, kT)):
                            ps = psum.tile([128, SC], F32, tag="mm",
                                           name=f"psqk_{b}_{tcn}_{hl}_{id(dst)}")
                            for d in range(16):
                                nc.tensor.matmul(
                                    ps[:], w_t[:, hl, d], xt[:, d],
                                    start=(d == 0), stop=(d == 15),
                                )
                            nc.vector.tensor_scalar_add(
                                dst[:, u, tcn * SC : (tcn + 1) * SC],
                                ps[:], b_t[:, hl],
                            )
                    for ts in range(SC // 128):
                        ps = psum.tile([128, SC], F32, tag="mm",
                                       name=f"psv_{b}_{tcn}_{ts}")
                        for d in range(16):
                            nc.tensor.matmul(
                                ps[:, : HL * 128],
                                xt[:, d, ts * 128 : (ts + 1) * 128],
                                wv_t[:, d],
                                start=(d == 0), stop=(d == 15),
                            )
                        tt_idx = tcn * (SC // 128) + ts
                        for hl in range(HL):
                            u = b * HL + hl
                            nc.vector.tensor_copy(
                                vv[:, u, tt_idx, 0:128],
                                ps[:, hl * 128 : (hl + 1) * 128],
                            )

            def attention_unit(u):
                b, hl = u // HL, u % HL
                for scn in range(S // SC):
                    o2 = [
                        psum.tile([128, 2, 129], F32, tag="o2", bufs=2,
                                  name=f"o2_{u}_{scn}_{i}")
                        for i in range(2)
                    ]
                    for tt in range(4 * scn + 4):
                        # causal trim: only s-subtiles >= tt within this chunk
                        off = max(0, tt - 4 * scn)  # first live s-subtile
                        nlive = 4 - off
                        s0 = scn * SC + off * 128
                        sp = psum.tile([128, SC], F32, tag="mm",
                                       name=f"sp_{u}_{scn}_{tt}")
                        nc.tensor.matmul(
                            sp[:, : nlive * 128],
                            kT[:, u, tt * 128 : (tt + 1) * 128],
                            qT[:, u, s0 : (scn + 1) * SC],
                            start=True, stop=True,
                        )
                        ex = expp.tile([128, SC], BF16, tag="ex",
                                       name=f"ex_{u}_{scn}_{tt}")
                        nc.scalar.activation(
                            ex[:, : nlive * 128], sp[:, : nlive * 128],
                            mybir.ActivationFunctionType.Exp,
                            scale=INV_SQRT_DH,
                        )
                        if tt >= 4 * scn:  # diagonal sub-block: causal mask
                            nc.vector.tensor_mul(
                                ex[:, 0:128], ex[:, 0:128], mask_t[:]
                            )
                        for ss in range(off, 4):
                            st = 4 * scn + ss
                            # start=True clears has_written BANK-wide; only
                            # the first matmul touching each o2 bank may set
                            # it. The sibling slice's first write relies on
                            # the cleared has_written bits (overwrite mode).
                            nc.tensor.matmul(
                                o2[ss // 2][:, ss % 2, :],
                                ex[:, (ss - off) * 128 : (ss - off + 1) * 128],
                                vv[:, u, tt],
                                start=(tt == 0 and ss % 2 == 0),
                                stop=(tt == st),
                            )
                    for ss in range(4):
                        st = 4 * scn + ss
                        o2t = o2[ss // 2]
                        rc = small.tile([128, 1], F32, tag="rc",
                                        name=f"rc_{u}_{scn}_{ss}")
                        nc.vector.reciprocal(rc[:], o2t[:, ss % 2, 128:129])
                        an = small.tile([128, 128], BF16, tag="an",
                                        name=f"an_{u}_{scn}_{ss}")
                        nc.vector.tensor_scalar_mul(
                            an[:], o2t[:, ss % 2, 0:128], rc[:]
                        )
                        tp = psum.tile([128, 128], BF16, tag="tp", bufs=2,
                                       name=f"tp_{u}_{scn}_{ss}")
                        nc.tensor.transpose(tp[:], an[:], id_t[:])
                        at = small.tile([128, 128], BF16, tag="at",
                                        name=f"at_{u}_{scn}_{ss}")
                        nc.vector.tensor_scalar_add(at[:], tp[:], bv_t[:, hl])
                        nc.gpsimd.dma_start(a2a_in[b][st // 2, hl, st % 2], at[:])

            def load_rcv(b, rcv):
                for dt in range(16):
                    srcc, shl = dt // HL, dt % HL
                    nc.scalar.dma_start(
                        rcv[:, dt],
                        a2a_out[b][srcc, shl].rearrange("s p m -> p s m"),
                    )

            def proj_phase(b, rcv):
                for oc in range(16):
                    ps = psum.tile([128, TOKB], F32, tag="mm",
                                   name=f"pso_{b}_{oc}")
                    for dt in range(16):
                        nc.tensor.matmul(
                            ps[:],
                            wo_t[:, dt, oc * 128 : (oc + 1) * 128],
                            rcv[:, dt],
                            start=(dt == 0), stop=(dt == 15),
                        )
                    ot = outp.tile([128, TOKB], F32, tag="ot",
                                   name=f"ot_{b}_{oc}")
                    nc.vector.tensor_copy(ot[:], ps[:])
                    nc.sync.dma_start(out[b, oc * 128 : (oc + 1) * 128, :], ot[:])

            # ---------------- program order ----------------
            qkv_phase(0)
            attention_unit(0)
            attention_unit(1)
            nc.gpsimd.collective_compute(
                "AllToAll",
                mybir.AluOpType.bypass,
                ins=[a2a_in[0].opt()],
                outs=[a2a_out[0].opt()],
                replica_groups=[list(range(NC))],
            )
            # W_out load: big (8.4 MB); emit after b0 attention so it does not
            # delay the startup-critical DMAs, well before the projection.
            wo_t = wopool.tile([128, 16, D], BF16, tag="wo")
            nc.scalar.dma_start(wo_t[:], w_out[:].rearrange("d p m -> p d m"))
            # rcv0 loads fire as soon as the b0 collective lands (during b1
            # QKV/attention), so proj(b0) can fill the b1-collective bubble.
            rcv0 = rcvpool.tile([128, 16, TOKB], BF16, tag="rcv0")
            load_rcv(0, rcv0)
            qkv_phase(1)
            attention_unit(2)
            attention_unit(3)
            nc.gpsimd.collective_compute(
                "AllToAll",
                mybir.AluOpType.bypass,
                ins=[a2a_in[1].opt()],
                outs=[a2a_out[1].opt()],
                replica_groups=[list(range(NC))],
            )
            proj_phase(0, rcv0)
            rcv1 = rcvpool.tile([128, 16, TOKB], BF16, tag="rcv1")
            load_rcv(1, rcv1)
            proj_phase(1, rcv1)
            if debug:
                nc.sync.dma_start(dbg_qT[:], qT[:])
                nc.sync.dma_start(dbg_kT[:], kT[:])
                nc.sync.dma_start(dbg_vv[:], vv[:])
                nc.sync.dma_start(dbg_a2a[:], a2a_in[0][:])

    nc.compile()
    return nc


def make_in_maps(x, W_in, b_in, W_out, b_out):
    xT = np.ascontiguousarray(x.transpose(0, 2, 1)).astype(NPBF16)  # [B, D, S]
    ident = np.eye(128, dtype=NPBF16)
    maskp = np.triu(np.ones((128, 128), dtype=np.float32)).astype(NPBF16)
    w_out_t = np.ascontiguousarray(W_out.reshape(16, 128, D)).astype(NPBF16)

    in_maps = []
    for c in range(NC):
        hs = [2 * c + hl for hl in range(HL)]  # global head ids
        wq_c = np.stack(
            [W_in[:, h * 128 : (h + 1) * 128].reshape(16, 128, 128) for h in hs]
        ).astype(NPBF16)
        wk_c = np.stack(
            [W_in[:, D + h * 128 : D + (h + 1) * 128].reshape(16, 128, 128) for h in hs]
        ).astype(NPBF16)
        wv_c = np.concatenate(
            [
                W_in[:, 2 * D + h * 128 : 2 * D + (h + 1) * 128].reshape(16, 128, 128)
                for h in hs
            ],
            axis=2,
        ).astype(NPBF16)
        bq_c = np.stack([b_in[h * 128 : (h + 1) * 128] for h in hs]).reshape(
            HL, 128, 1
        ).astype(np.float32)
        bk_c = np.stack([b_in[D + h * 128 : D + (h + 1) * 128] for h in hs]).reshape(
            HL, 128, 1
        ).astype(np.float32)
        bv_c = np.stack(
            [b_in[2 * D + h * 128 : 2 * D + (h + 1) * 128] for h in hs]
        ).reshape(HL, 128, 1).astype(np.float32)
        in_maps.append(
            {
                "xT": xT,
                "wq": wq_c,
                "wk": wk_c,
                "wv": wv_c,
                "bq": bq_c,
                "bk": bk_c,
                "bv": bv_c,
                "w_out": w_out_t,
                "ident": ident,
                "maskp": maskp,
            }
        )
    return in_maps


_NC_CACHE = {}


def _get_nc(debug=False):
    key = f"nc{debug}"
    if key not in _NC_CACHE:
        _NC_CACHE[key] = build_kernel(debug)
    return _NC_CACHE[key]


def kernel(x, W_in, b_in, W_out, b_out, _trace=False, _debug=False, **kw):
    x = np.asarray(x, dtype=np.float32)
    W_in = np.asarray(W_in, dtype=np.float32)
    b_in = np.asarray(b_in, dtype=np.float32)
    W_out = np.asarray(W_out, dtype=np.float32)
    b_out = np.asarray(b_out, dtype=np.float32)

    nc = _get_nc(_debug)
    in_maps = make_in_maps(x, W_in, b_in, W_out, b_out)
    res = run_bass_kernel_spmd(nc, in_maps, core_ids=list(range(NC)), trace=_trace)
    outf = np.empty((B, S, D), dtype=np.float32)
    for c in range(NC):
        o = np.asarray(res.results[c]["out"])  # [B, D, TOKB]
        for b in range(B):
            outf[b, c * TOKB : (c + 1) * TOKB, :] = o[b].T
    outf += b_out[None, None, :]
    if _trace or _debug:
        return outf, res
    return outf

